# revision 37
# baseline (speedup 1.0000x reference)
"""NCN link predictor (nn_NCNPredictor_77292231459355) on 8 Trainium2 cores.

Strategy (B-sharded per the sharding hint): the 1024 target pairs are split
128 per core (the SBUF partition dim). The host symmetrizes edge_index and
re-shards it by target row — the natural CSR shard for a B-partition: each
core receives the padded adjacency rows of ITS 128 (i, j) target pairs,
with each pair's neighbor lists split into KB=8 value-contiguous buckets
whose max i/j load is minimized per pair (binary search + greedy split);
matching values land in the same bucket by construction. Neighbor entries
are remapped per pair to union ranks (< 2048, exact in f16) to halve the
eq-gating DMA. Inputs ship as three blocks on parallel DMA queues: tiny
`tin` (tij, f16 identity, host-gathered f16 x[tar_i]/x[tar_j] rows,
original j ids), `nin` (f16 bucketed rank lists — gates the eq), and `wts`
(b1, W2, f16 W1). On device, each core:
  1. computes c[b,k,q] = multiplicity of j-neighbor q in i's bucket-k list
     via one bucketed broadcast equality (KB*sbj*sbi compares instead of
     sj*si) + a bf16 halving tree (tensor_tensor adds hit the 2x DVE mode;
     TENSOR_REDUCE has no fast mode),
  2. compacts the (extremely sparse) nonzero weights with a top-8 pass on
     the key c*2^17 + node_id (pad slots encode to exactly 0, no clamp),
  3. indirect-DMA-gathers just the top rows of x (the ucode path is warmed
     by an early dummy gather), accumulating xcn = sum w_k*x[n_k] on DVE
     as each gather's completion semaphore lands,
  4. runs the MLP head with f16 PE transposes/matmuls (2x vs fp32
     LOW_HIGH; ~5e-4 rel err vs the 2e-2 gate), relu fused into the W2
     product on DVE, and writes the scores PE-transposed to one partition
     as a single contiguous 256B DMA line (a [128,1] partition-scattered
     write trickles 16 completion semaphores over ~7.5us).
Known HW traps encoded here: Pool rejects TensorTensor in codegen; PE
transpose ignores PSUM accumulate flags on HW (CoreSim models it).
Host concatenates the 8 per-core [128] score slices into the final [1024].
"""

import numpy as np

N_NODES = 100000
B = 1024
D = 128
DH = 512
N_CORES = 8
BL = B // N_CORES  # 128 pairs per core = SBUF partition dim
TOPK = 8
KB = 8  # neighbor-list value-range buckets per pair

_compiled_cache: dict = {}


def _neighbor_lists(src, dst, targets):
    """Per-target sorted neighbor arrays (with multiplicity) of the
    symmetric edge list."""
    b = targets.shape[0]
    pos = np.full(N_NODES, -1, np.int32)
    pos[targets] = np.arange(b, dtype=np.int32)
    r = pos[src]
    m = r >= 0
    rows = r[m].astype(np.int64)
    cols = dst[m].astype(np.int64)
    order = np.lexsort((cols, rows))
    rows = rows[order]
    cols = cols[order]
    cnt = np.bincount(rows, minlength=b)
    starts = np.zeros(b + 1, np.int64)
    np.cumsum(cnt, out=starts[1:])
    return [cols[starts[i] : starts[i + 1]] for i in range(b)]


def _bucketize(li_all, lj_all):
    """Split each pair's sorted i/j neighbor lists into KB value-range
    buckets balanced on the union count (boundaries never split equal
    values, so every cross-list match stays within one bucket).
    Entries are remapped per pair to their rank in the sorted unique union
    (< 2048, so exact in f16 — halves the eq-gating DMA); `njo` carries the
    original j node ids (for the top-k key) in the same slot layout.
    Returns (ni [B,KB,sbi], nj [B,KB,sbj], njo) with pads -1 / -2."""
    b = len(li_all)
    ibuckets, jbuckets = [], []
    sbi = sbj = 1

    def greedy_buckets(cvi, cvj, T):
        """Value-contiguous partition with per-bucket i/j loads <= T.
        Returns the bucket id per distinct value, or None if > KB buckets."""
        out = np.empty(len(cvi), np.int64)
        bk = 0
        si = sj = 0
        for v, (a, c) in enumerate(zip(cvi, cvj)):
            if si + a > T or sj + c > T:
                bk += 1
                si = sj = 0
                if bk >= KB:
                    return None
            si += a
            sj += c
            out[v] = bk
        return out

    for pb in range(b):
        li, lj = li_all[pb], lj_all[pb]
        uniq = np.unique(np.concatenate([li, lj]))
        li_r = np.searchsorted(uniq, li)
        lj_r = np.searchsorted(uniq, lj)
        nv = len(uniq)
        if nv:
            cvi = np.bincount(li_r, minlength=nv)
            cvj = np.bincount(lj_r, minlength=nv)
            # minimal max bucket load via binary search + greedy split
            lo = max(1, -(-len(li) // KB), -(-len(lj) // KB),
                     int(cvi.max()), int(cvj.max()))
            hi = max(len(li), len(lj), 1)
            best = None
            while lo < hi:
                mid = (lo + hi) // 2
                g = greedy_buckets(cvi, cvj, mid)
                if g is not None:
                    best = g
                    hi = mid
                else:
                    lo = mid + 1
            if best is None or hi > lo:
                best = greedy_buckets(cvi, cvj, lo)
            vb = best if best is not None else np.zeros(nv, np.int64)
            bi = vb[li_r]
            bj = vb[lj_r]
        else:
            bi = np.zeros(0, np.int64)
            bj = np.zeros(0, np.int64)
        ci = np.bincount(bi, minlength=KB)
        cj = np.bincount(bj, minlength=KB)
        sbi = max(sbi, int(ci.max()))
        sbj = max(sbj, int(cj.max()))
        ibuckets.append((li_r, ci))
        jbuckets.append((lj_r, lj, cj))
    sbi = max(4, (sbi + 3) // 4 * 4)
    sbj = max(4, (sbj + 3) // 4 * 4)
    ni = np.full((b, KB, sbi), -1.0, np.float32)
    nj = np.full((b, KB, sbj), -2.0, np.float32)
    njo = np.zeros((b, KB, sbj), np.float32)
    for pb in range(b):
        li_r, ci = ibuckets[pb]
        lj_r, lj, cj = jbuckets[pb]
        # lists are sorted, so bucket ids are non-decreasing: slices suffice
        oi = np.zeros(KB + 1, np.int64)
        np.cumsum(ci, out=oi[1:])
        oj = np.zeros(KB + 1, np.int64)
        np.cumsum(cj, out=oj[1:])
        for k in range(KB):
            ni[pb, k, : ci[k]] = li_r[oi[k] : oi[k + 1]]
            nj[pb, k, : cj[k]] = lj_r[oj[k] : oj[k + 1]]
            njo[pb, k, : cj[k]] = lj[oj[k] : oj[k + 1]]
    return ni, nj, njo, sbi, sbj


def _big_layout(sbi, sbj):
    """Column offsets for the three merged [128, W] per-core input blocks:
    `tin` (tiny, unblocks the x[tar] gathers + transposes), `nin` (gates the
    equality pass; shipped on a parallel DMA queue) and `wts` (weights,
    needed late)."""
    lay = {}
    off = 0
    for name, w in [("tij", 2), ("ident", BL // 2),
                    ("xia", D // 2), ("xja", D // 2), ("njo", KB * sbj)]:
        lay[name] = ("tin", off, w)
        off += w
    tin_w = off
    off = 0
    for name, w in [("ni", KB * sbi // 2), ("nj", KB * sbj // 2), ("b2b", 1)]:
        lay[name] = ("nin", off, w)
        off += w
    nin_w = off
    off = 0
    for name, w in [("b1b", DH), ("w2b", DH),
                    ("w1a", DH // 2), ("w1b", DH // 2)]:
        lay[name] = ("wts", off, w)
        off += w
    return lay, tin_w, nin_w, off


def _build_bass(sbi, sbj, total_slots, repeat=1):
    """repeat>1 unrolls the whole body N times over the same tiles (serial
    via WAW deps) — used only for amplified wall-clock timing."""
    import concourse.bass as bass
    import concourse.tile as tile
    from concourse import bacc, mybir

    f32 = mybir.dt.float32
    f16 = mybir.dt.float16
    bf16 = mybir.dt.bfloat16
    i32 = mybir.dt.int32

    lay, tinw, ninw, wtsw = _big_layout(sbi, sbj)
    sq = KB * sbj  # total j-slot count per pair

    nc = bacc.Bacc(
        "TRN2", target_bir_lowering=False, debug=False, num_devices=N_CORES
    )

    tin_d = nc.dram_tensor("tin", [BL, tinw], f32, kind="ExternalInput").ap()
    nin_d = nc.dram_tensor("nin", [BL, ninw], f32, kind="ExternalInput").ap()
    wts_d = nc.dram_tensor("wts", [BL, wtsw], f32, kind="ExternalInput").ap()
    x_d = nc.dram_tensor("x", [N_NODES, D], f32, kind="ExternalInput").ap()
    out_d = nc.dram_tensor("out", [1, BL], f16, kind="ExternalOutput").ap()

    with tile.TileContext(nc) as tc:
        with (
            tc.tile_pool(name="sb", bufs=2) as sb,
            tc.tile_pool(name="ps", bufs=2, space="PSUM") as ps,
        ):
          for _rep in range(repeat):
            tin = sb.tile([BL, tinw], f32, tag="tin")
            nc.sync.dma_start(tin[:], tin_d[:])
            # nin rides the Scalar engine's DMA queue, in parallel with
            # tin/wts on Sync's, so the eq pass isn't queued behind wts.
            # (One DMA: completion latency is ~fixed per transfer, so
            # splitting into sequential chunks only stacks trickles.)
            nin = sb.tile([BL, ninw], f32, tag="nin")
            nc.scalar.dma_start(nin[:], nin_d[:])
            wts = sb.tile([BL, wtsw], f32, tag="wts")
            nc.sync.dma_start(wts[:], wts_d[:])

            def bslice(name):
                blk, off, w = lay[name]
                t = {"tin": tin, "nin": nin, "wts": wts}[blk]
                return t[:, off : off + w]

            nif = bslice("ni").bitcast(f16)
            njf = bslice("nj").bitcast(f16)
            njo = bslice("njo")
            tij = bslice("tij").bitcast(i32)
            b2b = bslice("b2b")
            ident = bslice("ident")
            b1b = bslice("b1b")
            w2b = bslice("w2b")
            w1a = bslice("w1a")
            w1b = bslice("w1b")

            # --- warm the indirect-DMA ucode path with a dummy 1-row gather
            # (depends only on the memset, so it runs during the input DMAs
            # and absorbs the one-time ~9us cold start off the critical path)
            dumi = sb.tile([2, 1], i32, tag="dumi")
            nc.vector.memset(dumi[:], 0)
            dumx = sb.tile([2, D], f32, tag="dumx")
            nc.gpsimd.indirect_dma_start(
                out=dumx[:], out_offset=None, in_=x_d[:],
                in_offset=bass.IndirectOffsetOnAxis(ap=dumi[:, 0:1], axis=0),
            )

            # --- xij = x[tar_i] * x[tar_j]; the rows ride in `tin`
            # (host-gathered layout, f16) so no indirect DMA + completion
            # trickle sits on this path. First in the Vector queue.
            xs = sb.tile([BL, D], f16, tag="xs")
            nc.vector.tensor_mul(
                out=xs[:],
                in0=bslice("xia").bitcast(f16),
                in1=bslice("xja").bitcast(f16),
            )

            # --- bucketed intersection counts (one 4D DVE op):
            # c[b,k,q] = sum_i (NJ[b,k,q] == NI[b,k,i]); NB the Pool engine
            # rejects TensorTensor in walrus codegen (NCC_IXCG966), so the
            # eq pass cannot be split off the DVE
            eq3 = sb.tile([BL, sq * sbi], bf16, tag="eq3")

            nc.vector.tensor_tensor(
                out=eq3[:].rearrange("p (k q i) -> p k q i", q=sbj, i=sbi),
                in0=njf.rearrange("p (k q) -> p k q", q=sbj)
                    .unsqueeze(3).broadcast_to([BL, KB, sbj, sbi]),
                in1=nif.rearrange("p (k i) -> p k i", i=sbi)
                    .unsqueeze(2).broadcast_to([BL, KB, sbj, sbi]),
                op=mybir.AluOpType.is_equal,
            )
            # halving tree over the i axis: bf16 tensor_tensor adds run in
            # the 2x DVE mode (all operands 2-byte + packed), unlike the 1x
            # TENSOR_REDUCE. Counts <= sbi are exact in bf16.
            with nc.allow_low_precision("bf16 adds of small exact ints"):
                lvl = eq3
                w = sbi
                while w % 2 == 0 and w > 1:
                    h = w // 2
                    nxt = sb.tile([BL, sq * h], bf16, tag=f"tree_{h}")
                    va = lvl[:].rearrange("p (q i) -> p q i", q=sq)
                    nc.vector.tensor_tensor(
                        out=nxt[:].rearrange("p (q i) -> p q i", q=sq),
                        in0=va[:, :, 0:h],
                        in1=va[:, :, h:w],
                        op=mybir.AluOpType.add,
                    )
                    lvl = nxt
                    w = h
                if w > 1:
                    cmat = sb.tile([BL, sq], bf16, tag="cmat")
                    nc.vector.tensor_reduce(
                        out=cmat[:],
                        in_=lvl[:].rearrange("p (q i) -> p q i", q=sq),
                        axis=mybir.AxisListType.X,
                        op=mybir.AluOpType.add,
                    )
                else:
                    cmat = lvl

            # --- pack keys t = c*2^17 + nj, clamp pads to 0 ---
            tkey = sb.tile([BL, sq], f32, tag="tkey")
            nc.vector.scalar_tensor_tensor(
                out=tkey[:],
                in0=cmat[:],
                scalar=131072.0,
                in1=njo,
                op0=mybir.AluOpType.mult,
                op1=mybir.AluOpType.add,
            )

            # --- top-8 rounds: decode (w, n), gather x rows, accumulate.
            # Keys sort descending, so positive-weight slots occupy the first
            # `total_slots` columns globally; gather only those. ---
            n_rounds = max(1, -(-total_slots // TOPK))
            tk = tkey
            gathers = []
            w8fs = []
            for r in range(n_rounds):
                g = min(TOPK, max(1, total_slots) - r * TOPK)
                t8 = sb.tile([BL, 8], f32, tag=f"t8_{r}")
                nc.vector.max(out=t8[:], in_=tk[:])
                t8i = sb.tile([BL, 8], i32, tag=f"t8i_{r}")
                nc.vector.tensor_copy(out=t8i[:], in_=t8[:])
                n8i = sb.tile([BL, 8], i32, tag=f"n8i_{r}")
                nc.vector.tensor_single_scalar(
                    out=n8i[:], in_=t8i[:], scalar=131071,
                    op=mybir.AluOpType.bitwise_and,
                )
                for k in range(g):
                    xsel = sb.tile([BL, D], f32, tag=f"xsel_{r}_{k}")
                    nc.gpsimd.indirect_dma_start(
                        out=xsel[:], out_offset=None, in_=x_d[:],
                        in_offset=bass.IndirectOffsetOnAxis(
                            ap=n8i[:, k : k + 1], axis=0
                        ),
                    )
                    gathers.append((xsel, r, k))
                w8i = sb.tile([BL, 8], i32, tag=f"w8i_{r}")
                nc.vector.tensor_single_scalar(
                    out=w8i[:], in_=t8i[:], scalar=17,
                    op=mybir.AluOpType.arith_shift_right,
                )
                w8f = sb.tile([BL, 8], f32, tag=f"w8f_{r}")
                nc.vector.tensor_copy(out=w8f[:], in_=w8i[:])
                w8fs.append(w8f)
                if r + 1 < n_rounds:
                    tk2 = sb.tile([BL, sq], f32, tag=f"tkey_{r + 1}")
                    nc.vector.match_replace(
                        out=tk2[:], in_to_replace=t8[:], in_values=tk[:],
                        imm_value=0.0,
                    )
                    tk = tk2

            # --- MLP head: out = relu(xs @ W1 + b1) @ W2 + b2 ---
            # xs/W1/identity in f16 (11-bit mantissa, ~5e-4 rel err — far
            # inside the 2e-2 tolerance) for 2x PE rate vs fp32 LOW_HIGH
            identh = ident.bitcast(f16)
            pst0 = ps.tile([BL, BL], f16, tag="pst0")
            pst1 = ps.tile([BL, BL], f16, tag="pst1")
            nc.tensor.transpose(out=pst0[:], in_=xs[:], identity=identh)

            # xcn accumulates on DVE (each term as soon as its gather lands;
            # NB transpose-accumulate in PSUM passes CoreSim but the HW PE
            # ignores acc flags in transpose mode — keep the adds on DVE)
            xcn = sb.tile([BL, D], f16, tag="xcn")
            for gi, (xsel, r, k) in enumerate(gathers):
                w8f = w8fs[r]
                if gi == 0:
                    nc.vector.tensor_scalar_mul(
                        out=xcn[:], in0=xsel[:], scalar1=w8f[:, k : k + 1]
                    )
                else:
                    nc.vector.scalar_tensor_tensor(
                        out=xcn[:],
                        in0=xsel[:],
                        scalar=w8f[:, k : k + 1],
                        in1=xcn[:],
                        op0=mybir.AluOpType.mult,
                        op1=mybir.AluOpType.add,
                    )
            nc.tensor.transpose(out=pst1[:], in_=xcn[:], identity=identh)
            xst0 = sb.tile([BL, BL], f16, tag="xst0")
            xst1 = sb.tile([BL, BL], f16, tag="xst1")
            nc.scalar.copy(out=xst0[:], in_=pst0[:])
            nc.vector.tensor_copy(out=xst1[:], in_=pst1[:])

            psh = ps.tile([BL, DH], f32, tag="psh")
            nc.scalar.copy(out=psh[:], in_=b1b)
            nc.tensor.matmul(
                psh[:], lhsT=xst0[:], rhs=w1a.bitcast(f16),
                start=False, stop=False, skip_group_check=True,
            )
            nc.tensor.matmul(
                psh[:], lhsT=xst1[:], rhs=w1b.bitcast(f16),
                start=False, stop=True, skip_group_check=True,
            )
            # relu fused into the W2 multiply: (psh max 0) * w2, one DVE op
            scratch = sb.tile([BL, DH], f32, tag="scratch")
            nc.vector.scalar_tensor_tensor(
                out=scratch[:],
                in0=psh[:],
                scalar=0.0,
                in1=w2b,
                op0=mybir.AluOpType.max,
                op1=mybir.AluOpType.mult,
            )
            res = sb.tile([BL, 1], f32, tag="res")
            nc.vector.tensor_reduce(
                out=res[:],
                in_=scratch[:].rearrange("p (q i) -> p q i", q=1),
                axis=mybir.AxisListType.X,
                op=mybir.AluOpType.add,
            )
            res16 = sb.tile([BL, 1], f16, tag="res16")
            with nc.allow_low_precision("single f16 round of final scores"):
                nc.vector.tensor_scalar_add(
                    out=res16[:], in0=res[:], scalar1=b2b
                )

            # transpose scores to one partition so the output is a single
            # contiguous 256B DMA line (not a 128x scattered write)
            psr = ps.tile([1, BL], f16, tag="psr")
            nc.tensor.transpose(out=psr[:], in_=res16[:], identity=identh)
            rest = sb.tile([1, BL], f16, tag="rest")
            nc.vector.tensor_copy(out=rest[:], in_=psr[:])
            nc.gpsimd.dma_start(out_d[:], rest[:])

    nc.compile()
    return nc


def _prepare(x, edge_index, tar_ei, W1, b1, W2, b2):
    e0 = np.asarray(edge_index[0]).astype(np.int64)
    e1 = np.asarray(edge_index[1]).astype(np.int64)
    src = np.concatenate([e0, e1])
    dst = np.concatenate([e1, e0])
    tar_i = np.asarray(tar_ei[0]).astype(np.int64)
    tar_j = np.asarray(tar_ei[1]).astype(np.int64)

    li = _neighbor_lists(src, dst, tar_i)
    lj = _neighbor_lists(src, dst, tar_j)
    ni, nj, njo, sbi, sbj = _bucketize(li, lj)
    assert sbi <= 512 and sbj <= 512, (sbi, sbj)

    # Safety sizing: rounds of top-8 needed to cover every pair's count of
    # nonzero-weight j-slots (pure planning; the device recomputes all of it).
    eq = nj[:, :, :, None] == ni[:, :, None, :]
    total_slots = max(1, int(eq.any(-1).sum((-2, -1)).max()))

    x = np.ascontiguousarray(np.asarray(x, dtype=np.float32))
    w1 = np.asarray(W1, dtype=np.float32)
    tij = np.stack([tar_i, tar_j], axis=1).astype(np.int32)

    lay, tinw, ninw, wtsw = _big_layout(sbi, sbj)
    blocks = {"tin": np.zeros((B, tinw), np.float32),
              "nin": np.zeros((B, ninw), np.float32),
              "wts": np.zeros((B, wtsw), np.float32)}

    def put(name, val):
        blk, off, w = lay[name]
        blocks[blk][:, off : off + w] = val

    put("tij", tij.view(np.float32))
    put("xia", x[tar_i].astype(np.float16).view(np.float32))
    put("xja", x[tar_j].astype(np.float16).view(np.float32))
    put("ident", np.tile(np.eye(BL, dtype=np.float16), (N_CORES, 1))
        .reshape(B, BL // 2, 2).view(np.float32).reshape(B, BL // 2))
    put("ni", ni.reshape(B, KB * sbi).astype(np.float16).view(np.float32))
    put("nj", nj.reshape(B, KB * sbj).astype(np.float16).view(np.float32))
    put("njo", njo.reshape(B, KB * sbj))
    put("b2b", np.float32(np.asarray(b2).reshape(-1)[0]))
    put("b1b", np.asarray(b1, np.float32)[None, :])
    put("w2b", np.asarray(W2, np.float32).reshape(1, DH))
    w1h = w1.astype(np.float16)
    put("w1a", np.tile(w1h[0:D].reshape(D, DH // 2, 2).view(np.float32)
                       .reshape(D, DH // 2), (N_CORES, 1)))
    put("w1b", np.tile(w1h[D : 2 * D].reshape(D, DH // 2, 2).view(np.float32)
                       .reshape(D, DH // 2), (N_CORES, 1)))

    in_maps = []
    for ci in range(N_CORES):
        sl = slice(ci * BL, (ci + 1) * BL)
        in_maps.append({
            "tin": np.ascontiguousarray(blocks["tin"][sl]),
            "nin": np.ascontiguousarray(blocks["nin"][sl]),
            "wts": np.ascontiguousarray(blocks["wts"][sl]),
            "x": x,
        })
    return in_maps, sbi, sbj, total_slots


def kernel(x, edge_index, tar_ei, W1, b1, W2, b2):
    from concourse.bass_utils import run_bass_kernel_spmd

    in_maps, sbi, sbj, total_slots = _prepare(x, edge_index, tar_ei, W1, b1, W2, b2)

    key = (sbi, sbj, total_slots)
    if key not in _compiled_cache:
        _compiled_cache[key] = _build_bass(sbi, sbj, total_slots)
    nc = _compiled_cache[key]

    res = run_bass_kernel_spmd(nc, in_maps, list(range(N_CORES)))
    return np.concatenate(
        [res.results[ci]["out"].reshape(BL).astype(np.float32) for ci in range(N_CORES)]
    ).astype(np.float32)
